# revision 1
# baseline (speedup 1.0000x reference)
"""Cross-attention (GQA, key-padding + shifted-causal mask) on 8 Trainium2 cores.

Sharding: core k handles batch b = k//4 and kv heads {2*(k%4), 2*(k%4)+1}
(each with its 4 query heads under GQA) -> 8 (b,h) attention instances per
core, fully independent (no collectives).

Mask algebra: the reference adds -10000 for padded keys and *replaces* with
-10000 where s > t + len_b - Sk. Since len_b >= Sk/2, the causal condition
subsumes the padding one, so the effective rule is "key s visible to query t
iff s <= t - c_b" with c_b = Sk - len_b. Rolling K/V right by c_b turns this
into a standard causal mask (s' <= t), which is compile-time structure: the
same SPMD program works for any lengths. Rolled-in garbage rows (s' < c_b)
are neutralized by zeroed V rows and an m_pad-weighted denominator matmul.
Rows with t < c_b attend to nothing; the reference gives them a uniform
softmax (all scores equal -10000), i.e. mean(V) -- patched on host.

Per (b,h) the device computes, in score-transposed (ST) layout [s, t]:
  ST = (K'^T)^T @ Q^T      (f32r matmuls, 128-wide s blocks x 512-wide t)
  P  = exp(scale * ST)     (ScalarE, skipping blocks above the causal diag)
  P *= diag_mask           (only on the 4 diagonal block columns, VectorE)
  OT   = sum_s V'[s,d] P[s,t]        (PSUM accum over s blocks)
  den  = sum_s m_pad_rep[s,m] P[s,t] (same, gives den broadcast over m)
  out  = OT * 1/(den + eps)          (VectorE), stored d-major; host
                                      transposes back to (B, Sq, H, D).
"""

import numpy as np

B, SQ, SK, H, HK, D = 2, 2048, 2048, 32, 8, 128
G = H // HK            # query heads per kv head
N_CORES = 8
TQ = 512               # t (query) tile width
TS = 128               # s (key) tile width
NTQ = SQ // TQ         # 4 t-chunks
SCALE = 1.0 / float(np.sqrt(D))
DEN_EPS = 1e-30

_compiled = None


def _build_program():
    """Build + schedule the single SPMD Bass program (same for all cores)."""
    from contextlib import ExitStack
    import concourse.bass as bass
    import concourse.tile as tile
    from concourse import bacc, mybir

    f32 = mybir.dt.float32
    f32r = mybir.dt.float32r

    nc = bacc.Bacc("TRN2", target_bir_lowering=False, debug=False)
    qT_ap = nc.dram_tensor("qT", [2 * G, D, SQ], f32, kind="ExternalInput").ap()
    kT_ap = nc.dram_tensor("kT", [2, D, SK], f32, kind="ExternalInput").ap()
    v_ap = nc.dram_tensor("v", [2, TS, SK // TS * D], f32, kind="ExternalInput").ap()
    mpr_ap = nc.dram_tensor("mpr", [TS, SK], f32, kind="ExternalInput").ap()
    out_ap = nc.dram_tensor("out", [2 * G, D, SQ], f32, kind="ExternalOutput").ap()

    with tile.TileContext(nc) as tc, ExitStack() as ctx:
        const_pool = ctx.enter_context(tc.tile_pool(name="const", bufs=1))
        kv_pool = ctx.enter_context(tc.tile_pool(name="kv", bufs=2))
        q_pool = ctx.enter_context(tc.tile_pool(name="q", bufs=2))
        p_pool = ctx.enter_context(tc.tile_pool(name="p", bufs=4))
        fin_pool = ctx.enter_context(tc.tile_pool(name="fin", bufs=3))
        st_psum = ctx.enter_context(tc.tile_pool(name="st", bufs=2, space="PSUM"))
        ot_psum = ctx.enter_context(tc.tile_pool(name="ot", bufs=2, space="PSUM"))
        den_psum = ctx.enter_context(tc.tile_pool(name="den", bufs=2, space="PSUM"))

        mpr_sb = const_pool.tile([TS, SK], f32r)

        for ikv in range(2):
            kT_sb = kv_pool.tile([D, SK], f32r, tag="kT")
            v_sb = kv_pool.tile([TS, SK // TS * D], f32r, tag="v")
            if ikv == 0:
                # startup: order sync queue by first consumption, park the
                # not-immediately-needed loads on the idle gpsimd queue
                nc.sync.dma_start(kT_sb[:, :TQ], kT_ap[ikv][:, :TQ].bitcast(f32r))
                nc.gpsimd.dma_start(v_sb[:], v_ap[ikv].bitcast(f32r))
                nc.gpsimd.dma_start(mpr_sb[:], mpr_ap[:].bitcast(f32r))
            else:
                nc.sync.dma_start(kT_sb[:], kT_ap[ikv].bitcast(f32r))
                nc.sync.dma_start(v_sb[:], v_ap[ikv].bitcast(f32r))

            for j in range(G):
                ih = ikv * G + j
                qT_sb = q_pool.tile([D, SQ], f32r)
                if ikv == 0 and j == 0:
                    nc.sync.dma_start(qT_sb[:, :TQ], qT_ap[ih][:, :TQ].bitcast(f32r))
                    nc.sync.dma_start(qT_sb[:, TQ:], qT_ap[ih][:, TQ:].bitcast(f32r))
                    nc.sync.dma_start(kT_sb[:, TQ:], kT_ap[ikv][:, TQ:].bitcast(f32r))
                else:
                    nc.sync.dma_start(qT_sb[:], qT_ap[ih].bitcast(f32r))

                for t in range(NTQ):
                    n_sc = (TQ // TS) * (t + 1)  # causal: s blocks up to diag
                    ot_ps = ot_psum.tile([D, TQ], f32)
                    den_ps = den_psum.tile([TS, TQ], f32)
                    pending = None  # 1-deep SW pipeline keeps PE ahead of ACT
                    for pi in range(n_sc // 2):
                        sc0 = 2 * pi
                        st_ps = st_psum.tile([TS, 2 * TQ], f32)
                        for h in range(2):
                            nc.tensor.matmul(
                                st_ps[:, h * TQ : (h + 1) * TQ],
                                lhsT=kT_sb[:, (sc0 + h) * TS : (sc0 + h + 1) * TS],
                                rhs=qT_sb[:, t * TQ : (t + 1) * TQ],
                                start=True,
                                stop=True,
                            )
                        p_sb = p_pool.tile([TS, 2 * TQ], f32r)
                        nc.scalar.activation(
                            p_sb[:], st_ps[:],
                            mybir.ActivationFunctionType.Exp,
                            scale=SCALE,
                        )
                        for h in range(2):
                            o = sc0 + h - (n_sc - 4)
                            if o >= 0:  # diagonal block: causal pattern
                                nc.gpsimd.affine_select(
                                    out=p_sb[:, h * TQ : (h + 1) * TQ],
                                    in_=p_sb[:, h * TQ : (h + 1) * TQ],
                                    pattern=[[1, TQ]],
                                    compare_op=mybir.AluOpType.is_ge,
                                    fill=0.0,
                                    base=-o * TS,
                                    channel_multiplier=-1,
                                )
                        if pending is not None:
                            _pv_den(nc, pending, v_sb, mpr_sb, ot_ps, den_ps,
                                    first=(pending[0] == 0), last=False)
                        pending = (sc0, p_sb)
                    _pv_den(nc, pending, v_sb, mpr_sb, ot_ps, den_ps,
                            first=(pending[0] == 0), last=True,
                            n_sc=n_sc)

                    recip_sb = fin_pool.tile([TS, TQ], f32, tag="recip")
                    nc.vector.reciprocal_approx_fast(recip_sb[:], den_ps[:])
                    out_sb = fin_pool.tile([D, TQ], f32, tag="out")
                    nc.vector.tensor_tensor(
                        out=out_sb[:],
                        in0=ot_ps[:],
                        in1=recip_sb[:],
                        op=mybir.AluOpType.mult,
                    )
                    nc.sync.dma_start(
                        out_ap[ih][:, t * TQ : (t + 1) * TQ], out_sb[:]
                    )

    nc.compile()
    return nc


def _pv_den(nc, pending, v_sb, mpr_sb, ot_ps, den_ps, first, last, n_sc=None):
    sc0, p_sb = pending
    for h in range(2):
        sc = sc0 + h
        nc.tensor.matmul(
            ot_ps[:],
            lhsT=v_sb[:, sc * D : (sc + 1) * D],
            rhs=p_sb[:, h * TQ : (h + 1) * TQ],
            start=(first and h == 0),
            stop=(last and h == 1),
        )
        nc.tensor.matmul(
            den_ps[:],
            lhsT=mpr_sb[:, sc * TS : (sc + 1) * TS],
            rhs=p_sb[:, h * TQ : (h + 1) * TQ],
            start=(first and h == 0),
            stop=(last and h == 1),
        )


def _get_program():
    global _compiled
    if _compiled is None:
        _compiled = _build_program()
    return _compiled


def kernel(q, kv, key_padding_mask, _want_trace=False):
    q = np.asarray(q, dtype=np.float32)
    kv = np.asarray(kv, dtype=np.float32)
    mask = np.asarray(key_padding_mask).astype(bool)

    lengths = mask.sum(axis=1).astype(np.int64)  # valid keys per batch
    c = SK - lengths                             # roll shift per batch

    k_full = kv[:, :, 0]  # (B, SK, HK, D)
    v_full = kv[:, :, 1]

    # roll keys/values right by c[b]; only the first len_b keys are ever
    # visible so the tail [len_b:] is dropped. Pad region stays zero.
    k_roll = np.zeros_like(k_full)
    v_roll = np.zeros_like(v_full)
    for b in range(B):
        k_roll[b, c[b]:] = k_full[b, : lengths[b]]
        v_roll[b, c[b]:] = v_full[b, : lengths[b]]

    in_maps = []
    for core in range(N_CORES):
        b = core // 4
        hks = (2 * (core % 4), 2 * (core % 4) + 1)
        qT = np.empty((2 * G, D, SQ), dtype=np.float32)
        kT = np.empty((2, D, SK), dtype=np.float32)
        v_l = np.empty((2, TS, SK // TS * D), dtype=np.float32)
        for i, hk in enumerate(hks):
            kT[i] = k_roll[b, :, hk, :].T
            # v chunked: v_l[i][p, sc*D + d] = v_roll[b, sc*TS + p, hk, d]
            v_l[i] = np.ascontiguousarray(
                v_roll[b, :, hk, :].reshape(SK // TS, TS, D).transpose(1, 0, 2)
            ).reshape(TS, SK // TS * D)
            for j in range(G):
                qT[i * G + j] = q[b, :, hk * G + j, :].T
        mpad = (np.arange(SK) >= c[b]).astype(np.float32)
        # mpr[p, sc*TS + m] = mpad[sc*TS + p]  (column-replicated per chunk)
        mpr = np.repeat(
            mpad.reshape(SK // TS, TS, 1), TS, axis=2
        ).transpose(1, 0, 2).reshape(TS, SK).astype(np.float32)
        in_maps.append({
            "qT": np.ascontiguousarray(qT),
            "kT": np.ascontiguousarray(kT),
            "v": np.ascontiguousarray(v_l),
            "mpr": np.ascontiguousarray(mpr),
        })

    from concourse.bass_utils import run_bass_kernel_spmd

    nc = _get_program()
    res = run_bass_kernel_spmd(
        nc, in_maps, core_ids=list(range(N_CORES)),
        trace=_want_trace,
    )

    out = np.empty((B, SQ, H, D), dtype=np.float32)
    for core in range(N_CORES):
        b = core // 4
        hks = (2 * (core % 4), 2 * (core % 4) + 1)
        o_core = res.results[core]["out"]  # (2*G, D, SQ)
        for i, hk in enumerate(hks):
            for j in range(G):
                out[b, :, hk * G + j, :] = o_core[i * G + j].T

    # rows that attend to nothing: reference softmax is uniform -> mean(V)
    for b in range(B):
        if c[b] > 0:
            vm = v_full[b].mean(axis=0)  # (HK, D)
            out[b, : c[b]] = np.repeat(vm, G, axis=0)[None]

    if _want_trace:
        return out, res
    return out

